# revision 1
# baseline (speedup 1.0000x reference)
"""MoE-routing model (embed -> hebbian-routed 2-expert FFN -> tied-vocab logits)
as a Bass/Tile kernel on 8 TRN2 NeuronCores.

Sharding: router runs on host (top-k expert selection only picks which weights
are staged); tokens are sharded 8-ways (256 tokens/core). Each core runs:
  gather(tok_emb) + pos  ->  LayerNorm  ->  gelu(xn @ W1sel + b1) @ W2sel*rw
  + residual  ->  logits = combined @ tok_emb.T          (full vocab per core)
Host concatenates the per-core token slices of the logits.

All GEMMs run in bf16 with fp32 PSUM accumulation. LayerNorm gain/bias are
folded exactly into W1/b1 on the host; expert routing weights are folded into
W2; b2 is folded into the residual path via pos_resid = pos + sum_k rw_k*b2_k.
"""

import numpy as np
from contextlib import ExitStack

import ml_dtypes
import concourse.bass as bass
import concourse.mybir as mybir
import concourse.tile as tile
from concourse import bacc
from concourse.bass import ds, ts
from concourse.bass_utils import run_bass_kernel_spmd
from concourse.kernels.tile_matmul import (
    composable_matmul_tile_kernel,
    dma_from_dram_kxm,
    dma_from_dram_kxn,
    dma_to_dram_mxn,
    scalar_copyback,
)
from concourse.masks import make_identity

V, H, E, TOPK, I, R, S, B = 32000, 1024, 16, 2, 4096, 512, 1024, 2
NCORES = 8
N_TOK = B * S          # 2048
T = N_TOK // NCORES    # 256 tokens per core
KI = TOPK * I          # 8192 stacked expert intermediate dim

F32 = mybir.dt.float32
BF16 = mybir.dt.bfloat16
I32 = mybir.dt.int32
BF16_NP = ml_dtypes.bfloat16


def _gemm(ctx, tc, kxm_ap, kxn_ap, mxn_ap, *, output_type,
          transpose_kxm=False, reducer=None, consumer=None,
          kxm_bufs=4, kxn_bufs=4, tag=""):
    """Thin wrapper over composable_matmul_tile_kernel with plain DMA
    producers (all data is staged pre-cast in DRAM)."""
    nc = tc.nc
    kxm_pool = ctx.enter_context(tc.tile_pool(name=f"kxm_{tag}", bufs=kxm_bufs))
    kxn_pool = ctx.enter_context(tc.tile_pool(name=f"kxn_{tag}", bufs=kxn_bufs))
    kxm_producer, kxm_shape = dma_from_dram_kxm(
        kxm_pool, kxm_ap, transpose_ap=transpose_kxm)
    kxn_producer, kxn_shape = dma_from_dram_kxn(kxn_pool, kxn_ap)
    if consumer is None:
        consumer = dma_to_dram_mxn(mxn_ap)
    composable_matmul_tile_kernel(
        tc=tc,
        kxm_shape=kxm_shape,
        kxn_shape=kxn_shape,
        output_type=output_type,
        kxm_producer=kxm_producer,
        kxn_producer=kxn_producer,
        mxn_consumer=consumer,
        mxn_subtile_reducer=reducer if reducer is not None else scalar_copyback(),
    )


def build_kernel():
    nc = bacc.Bacc("TRN2", target_bir_lowering=False, debug=False)

    # ---- external inputs (per-core shards staged by host) ----
    ids = nc.dram_tensor("ids", [128, 2], I32, kind="ExternalInput")
    gtab = nc.dram_tensor("gtab", [V, H], BF16, kind="ExternalInput")
    pos_a = nc.dram_tensor("pos_a", [T, H], BF16, kind="ExternalInput")
    pos_r = nc.dram_tensor("pos_r", [T, H], BF16, kind="ExternalInput")
    w1a = nc.dram_tensor("w1a", [H, KI], BF16, kind="ExternalInput")
    b1a = nc.dram_tensor("b1a", [128, KI // 128], F32, kind="ExternalInput")
    w2a = nc.dram_tensor("w2a", [KI, H], BF16, kind="ExternalInput")
    temT = nc.dram_tensor("temT", [H, V], BF16, kind="ExternalInput")
    logits = nc.dram_tensor("logits", [T, V], F32, kind="ExternalOutput")

    # ---- internal DRAM scratch ----
    xnT = nc.dram_tensor("xnT", [H, T], BF16)
    hres = nc.dram_tensor("hres", [T, H], F32)
    interT = nc.dram_tensor("interT", [KI, T], BF16)
    comb = nc.dram_tensor("comb", [T, H], BF16)

    with tile.TileContext(nc) as tc, ExitStack() as octx:
        const = octx.enter_context(tc.tile_pool(name="const", bufs=1))
        ident = const.tile([128, 128], BF16)
        make_identity(nc, ident)
        eps_t = const.tile([128, 1], F32)
        nc.any.memset(eps_t[:], 1e-5)
        b1_sb = const.tile([128, KI // 128], F32)
        nc.sync.dma_start(b1_sb[:], b1a[:])
        ids_t = const.tile([128, 2], I32)
        nc.sync.dma_start(ids_t[:], ids[:])

        # ---------- Phase A: embedding gather + LayerNorm + xn transpose ----
        with ExitStack() as actx:
            emb = actx.enter_context(tc.tile_pool(name="emb", bufs=2))
            psA = actx.enter_context(tc.tile_pool(name="psA", bufs=2, space="PSUM"))
            for j in range(T // 128):
                g_t = emb.tile([128, H], BF16, name="g_t")
                nc.gpsimd.indirect_dma_start(
                    out=g_t[:], out_offset=None,
                    in_=gtab[:],
                    in_offset=bass.IndirectOffsetOnAxis(ap=ids_t[:, j:j + 1], axis=0),
                )
                pa_t = emb.tile([128, H], BF16, name="pa_t")
                nc.sync.dma_start(pa_t[:], pos_a[ts(j, 128), :])
                pr_t = emb.tile([128, H], BF16, name="pr_t")
                nc.sync.dma_start(pr_t[:], pos_r[ts(j, 128), :])

                h_t = emb.tile([128, H], F32, name="h_t")
                nc.vector.tensor_add(out=h_t[:], in0=g_t[:], in1=pa_t[:])
                hr_t = emb.tile([128, H], F32, name="hr_t")
                nc.vector.tensor_add(out=hr_t[:], in0=g_t[:], in1=pr_t[:])
                nc.sync.dma_start(hres[ts(j, 128), :], hr_t[:])

                # LayerNorm statistics (biased var = E[x^2] - E[x]^2)
                sum_t = emb.tile([128, 1], F32, name="sum_t")
                nc.vector.reduce_sum(out=sum_t[:], in_=h_t[:], axis=mybir.AxisListType.X)
                mean_t = emb.tile([128, 1], F32, name="mean_t")
                nc.vector.tensor_scalar_mul(mean_t[:], sum_t[:], 1.0 / H)
                sq_t = emb.tile([128, H], F32, name="sq_t")
                nc.vector.tensor_tensor(out=sq_t[:], in0=h_t[:], in1=h_t[:],
                                        op=mybir.AluOpType.mult)
                ss_t = emb.tile([128, 1], F32, name="ss_t")
                nc.vector.reduce_sum(out=ss_t[:], in_=sq_t[:], axis=mybir.AxisListType.X)
                ex2_t = emb.tile([128, 1], F32, name="ex2_t")
                nc.vector.tensor_scalar_mul(ex2_t[:], ss_t[:], 1.0 / H)
                msq_t = emb.tile([128, 1], F32, name="msq_t")
                nc.vector.tensor_tensor(out=msq_t[:], in0=mean_t[:], in1=mean_t[:],
                                        op=mybir.AluOpType.mult)
                var_t = emb.tile([128, 1], F32, name="var_t")
                nc.vector.tensor_tensor(out=var_t[:], in0=ex2_t[:], in1=msq_t[:],
                                        op=mybir.AluOpType.subtract)
                std_t = emb.tile([128, 1], F32, name="std_t")
                nc.scalar.activation(std_t[:], var_t[:],
                                     mybir.ActivationFunctionType.Sqrt,
                                     bias=eps_t[:, 0:1])
                rst_t = emb.tile([128, 1], F32, name="rst_t")
                nc.vector.reciprocal(rst_t[:], std_t[:])

                xn_t = emb.tile([128, H], BF16, name="xn_t")
                nc.vector.tensor_scalar(
                    out=xn_t[:], in0=h_t[:],
                    scalar1=mean_t[:, 0:1], scalar2=rst_t[:, 0:1],
                    op0=mybir.AluOpType.subtract, op1=mybir.AluOpType.mult)

                # transpose xn [128tok, H] -> xnT [H, 128tok]
                for kt in range(H // 128):
                    p_t = psA.tile([128, 128], BF16, name="p_t")
                    nc.tensor.transpose(p_t[:], xn_t[:, ts(kt, 128)], ident[:])
                    s_t = emb.tile([128, 128], BF16, name="s_t")
                    nc.any.tensor_copy(out=s_t[:], in_=p_t[:])
                    nc.sync.dma_start(xnT[ts(kt, 128), ts(j, 128)], s_t[:])

        # ---------- Phase B: interT = gelu(w1a.T @ xnT + b1) -----------------
        def g1_reducer(nc_, psum, sbuf, md):
            gi = md.m_tile_idx * md.m_subtiles + md.m_subtile_idx
            out2 = sbuf.rearrange("p a b -> p (a b)")
            nc_.scalar.activation(out2, psum[:, :out2.shape[1]],
                                  mybir.ActivationFunctionType.Gelu_apprx_tanh,
                                  bias=b1_sb[:, gi:gi + 1])

        with ExitStack() as bctx:
            _gemm(bctx, tc, w1a[:], xnT[:], interT[:], output_type=BF16,
                  reducer=g1_reducer, kxm_bufs=4, kxn_bufs=3, tag="g1")

        # ---------- Phase C: comb = interT.T @ w2a + hres --------------------
        hres3 = hres[:].rearrange("(po pi) f -> pi po f", pi=128)  # [128, 2, H]

        with ExitStack() as cctx:
            hrpool = cctx.enter_context(tc.tile_pool(name="hrpool", bufs=3))

            def g2_reducer(nc_, psum, sbuf, md):
                blk = md.m_tile_idx * md.m_subtiles + md.m_subtile_idx
                noff = md.n_tile_idx * md.n_tile + md.n_subtile_idx * md.n_subtile
                nsz = md.n_subtile
                hr = hrpool.tile([128, nsz], F32, name="hr")
                nc_.sync.dma_start(hr[:], hres3[:, blk, ds(noff, nsz)])
                out2 = sbuf.rearrange("p a b -> p (a b)")
                nc_.vector.tensor_tensor(out=out2, in0=psum[:, :nsz], in1=hr[:],
                                         op=mybir.AluOpType.add)

            _gemm(cctx, tc, interT[:], w2a[:], comb[:], output_type=BF16,
                  reducer=g2_reducer, kxm_bufs=17, kxn_bufs=4, tag="g2")

        # ---------- Phase D: logits = comb @ temT ---------------------------
        with ExitStack() as dctx:
            _gemm(dctx, tc, comb[:], temT[:], logits[:], output_type=F32,
                  transpose_kxm=True, kxm_bufs=3, kxn_bufs=6, tag="g3")

    nc.compile()
    return nc


_NC_CACHE = None


def _get_nc():
    global _NC_CACHE
    if _NC_CACHE is None:
        _NC_CACHE = build_kernel()
    return _NC_CACHE


def _softmax(x):
    e = np.exp(x - x.max())
    return e / e.sum()


def prepare_in_maps(inputs):
    ids = np.asarray(inputs["input_ids"]).astype(np.int32)          # [B, S]
    tok = np.asarray(inputs["tok_emb"], dtype=np.float32)           # [V, H]
    pos = np.asarray(inputs["pos_emb"], dtype=np.float32)[:S]       # [S, H]
    w_router_in = np.asarray(inputs["w_router_in"], dtype=np.float32)
    w_heb = np.asarray(inputs["w_heb"], dtype=np.float32)
    ln_g = np.asarray(inputs["ln_g"], dtype=np.float32)
    ln_b = np.asarray(inputs["ln_b"], dtype=np.float32)
    w1 = np.asarray(inputs["w1"], dtype=np.float32)
    b1 = np.asarray(inputs["b1"], dtype=np.float32)
    w2 = np.asarray(inputs["w2"], dtype=np.float32)
    b2 = np.asarray(inputs["b2"], dtype=np.float32)

    # ---- host-side Hebbian router: selects which expert weights to stage ----
    ids_flat = ids.reshape(-1)                                      # [N_TOK]
    h_mean = tok[ids_flat].mean(axis=0, dtype=np.float64).astype(np.float32)
    h_mean = h_mean + pos.mean(axis=0, dtype=np.float64).astype(np.float32)
    feat = np.tanh(h_mean @ w_router_in)                            # [R]
    scores = w_heb @ feat                                           # [E]
    top_idx = np.argsort(-scores, kind="stable")[:TOPK]
    rw = _softmax(scores[top_idx].astype(np.float32))               # [TOPK]

    # ---- fold LN gain into W1, LN bias into b1, routing weight into W2,
    #      routed b2 into the residual (via pos_resid) ----
    w1s = np.concatenate([ln_g[:, None] * w1[e] for e in top_idx], axis=1)  # [H, KI]
    b1s = np.concatenate([b1[e] + ln_b @ w1[e] for e in top_idx])           # [KI]
    w2s = np.concatenate([rw[k] * w2[e] for k, e in enumerate(top_idx)], axis=0)  # [KI, H]
    bias2 = sum(rw[k] * b2[e] for k, e in enumerate(top_idx))               # [H]

    gtab = tok.astype(BF16_NP)
    temT = np.ascontiguousarray(tok.T).astype(BF16_NP)
    pos_bf = pos.astype(BF16_NP)
    posr_bf = (pos + bias2[None, :]).astype(BF16_NP)
    w1a = w1s.astype(BF16_NP)
    w2a = w2s.astype(BF16_NP)
    b1a = np.ascontiguousarray(b1s.reshape(KI // 128, 128).T)       # [128, KI//128]

    in_maps = []
    for c in range(NCORES):
        tok_slice = ids_flat[c * T:(c + 1) * T]                     # [T]
        ids_pc = np.ascontiguousarray(tok_slice.reshape(T // 128, 128).T)  # [128, 2]
        s0 = (c * T) % S
        in_maps.append({
            "ids": ids_pc,
            "gtab": gtab,
            "pos_a": pos_bf[s0:s0 + T],
            "pos_r": posr_bf[s0:s0 + T],
            "w1a": w1a,
            "b1a": b1a,
            "w2a": w2a,
            "temT": temT,
        })
    return in_maps


def kernel(**inputs) -> np.ndarray:
    nc = _get_nc()
    in_maps = prepare_in_maps(inputs)
    res = run_bass_kernel_spmd(nc, in_maps, core_ids=list(range(NCORES)))
    parts = [np.asarray(res.results[c]["logits"]) for c in range(NCORES)]
    return np.concatenate(parts, axis=0).reshape(B, S, V).astype(np.float32)


# revision 6
# speedup vs baseline: 1.1452x; 1.1452x over previous
"""MoE-routing model (embed -> hebbian-routed 2-expert FFN -> tied-vocab logits)
as a Bass/Tile kernel on 8 TRN2 NeuronCores.

Sharding: router runs on host (top-k expert selection only picks which weights
are staged); tokens are sharded 8-ways (256 tokens/core). Each core runs:
  gather(tok_emb) + pos  ->  LayerNorm  ->  gelu(xn @ W1sel + b1) @ W2sel*rw
  + residual  ->  logits = combined @ tok_emb.T          (full vocab per core)
Host concatenates the per-core token slices of the logits.

All GEMMs run in bf16 with fp32 PSUM accumulation. LayerNorm gain/bias are
folded exactly into W1/b1 on the host; expert routing weights are folded into
W2; b2 is folded into the residual path via pos_resid = pos + sum_k rw_k*b2_k.
"""

import numpy as np
from contextlib import ExitStack

import ml_dtypes
import concourse.bass as bass
import concourse.mybir as mybir
import concourse.tile as tile
from concourse import bacc
from concourse.bass import ds, ts
from concourse.bass_utils import run_bass_kernel_spmd
from concourse.kernels.tile_matmul import (
    composable_matmul_tile_kernel,
    dma_from_dram_kxm,
    dma_from_dram_kxn,
    dma_to_dram_mxn,
    scalar_copyback,
)
from concourse.masks import make_identity

V, H, E, TOPK, I, R, S, B = 32000, 1024, 16, 2, 4096, 512, 1024, 2
NCORES = 8
N_TOK = B * S          # 2048
T = N_TOK // NCORES    # 256 tokens per core
KI = TOPK * I          # 8192 stacked expert intermediate dim

F32 = mybir.dt.float32
BF16 = mybir.dt.bfloat16
I32 = mybir.dt.int32
BF16_NP = ml_dtypes.bfloat16


def _vector_copyback(nc, psum, sbuf, md):
    # PSUM->SBUF eviction on DVE (idle in this kernel) instead of ACT.
    out2 = sbuf.rearrange("p a b -> p (a b)")
    nc.vector.tensor_copy(out=out2, in_=psum[:, :out2.shape[1]])


def _gemm(ctx, tc, kxm_ap, kxn_ap, mxn_ap, *, output_type,
          transpose_kxm=False, reducer=None, consumer=None,
          kxm_bufs=4, kxn_bufs=4, psum_bufs=2, temps_bufs=3, tag=""):
    """Thin wrapper over composable_matmul_tile_kernel with plain DMA
    producers (all data is staged pre-cast in DRAM)."""
    nc = tc.nc
    kxm_pool = ctx.enter_context(tc.tile_pool(name=f"kxm_{tag}", bufs=kxm_bufs))
    kxn_pool = ctx.enter_context(tc.tile_pool(name=f"kxn_{tag}", bufs=kxn_bufs))
    kxm_producer, kxm_shape = dma_from_dram_kxm(
        kxm_pool, kxm_ap, transpose_ap=transpose_kxm)
    kxn_producer, kxn_shape = dma_from_dram_kxn(kxn_pool, kxn_ap)
    if consumer is None:
        consumer = dma_to_dram_mxn(mxn_ap)
    composable_matmul_tile_kernel(
        tc=tc,
        kxm_shape=kxm_shape,
        kxn_shape=kxn_shape,
        output_type=output_type,
        kxm_producer=kxm_producer,
        kxn_producer=kxn_producer,
        mxn_consumer=consumer,
        mxn_subtile_reducer=reducer if reducer is not None else _vector_copyback,
        psum_n_bufs=psum_bufs,
        temps_n_bufs=temps_bufs,
    )


def build_kernel():
    nc = bacc.Bacc("TRN2", target_bir_lowering=False, debug=False)

    # ---- external inputs (per-core shards staged by host) ----
    ids = nc.dram_tensor("ids", [128, 2], I32, kind="ExternalInput")
    gtab = nc.dram_tensor("gtab", [V, H], BF16, kind="ExternalInput")
    pos_a = nc.dram_tensor("pos_a", [T, H], BF16, kind="ExternalInput")
    pos_r = nc.dram_tensor("pos_r", [T, H], BF16, kind="ExternalInput")
    w1a = nc.dram_tensor("w1a", [H, KI], BF16, kind="ExternalInput")
    b1a = nc.dram_tensor("b1a", [128, KI // 128], F32, kind="ExternalInput")
    w2a = nc.dram_tensor("w2a", [KI, H], BF16, kind="ExternalInput")
    temT = nc.dram_tensor("temT", [H, V], BF16, kind="ExternalInput")
    logits = nc.dram_tensor("logits", [T, V], BF16, kind="ExternalOutput")

    # ---- internal DRAM scratch ----
    xnT = nc.dram_tensor("xnT", [H, T], BF16)
    hres = nc.dram_tensor("hres", [T, H], F32)
    interT = nc.dram_tensor("interT", [KI, T], BF16)
    comb = nc.dram_tensor("comb", [T, H], BF16)

    with tile.TileContext(nc) as tc, ExitStack() as octx:
        const = octx.enter_context(tc.tile_pool(name="const", bufs=1))
        ident = const.tile([128, 128], BF16)
        make_identity(nc, ident)
        eps_t = const.tile([128, 1], F32)
        nc.any.memset(eps_t[:], 1e-5)
        b1_sb = const.tile([128, KI // 128], F32)
        nc.sync.dma_start(b1_sb[:], b1a[:])
        ids_t = const.tile([128, 2], I32)
        nc.sync.dma_start(ids_t[:], ids[:])

        # ---------- Phase A: embedding gather + LayerNorm + xn transpose ----
        with ExitStack() as actx:
            emb = actx.enter_context(tc.tile_pool(name="emb", bufs=2))
            psA = actx.enter_context(tc.tile_pool(name="psA", bufs=2, space="PSUM"))
            for j in range(T // 128):
                g_t = emb.tile([128, H], BF16, name="g_t")
                nc.gpsimd.indirect_dma_start(
                    out=g_t[:], out_offset=None,
                    in_=gtab[:],
                    in_offset=bass.IndirectOffsetOnAxis(ap=ids_t[:, j:j + 1], axis=0),
                )
                pa_t = emb.tile([128, H], BF16, name="pa_t")
                nc.sync.dma_start(pa_t[:], pos_a[ts(j, 128), :])
                pr_t = emb.tile([128, H], BF16, name="pr_t")
                nc.sync.dma_start(pr_t[:], pos_r[ts(j, 128), :])

                h_t = emb.tile([128, H], F32, name="h_t")
                nc.vector.tensor_add(out=h_t[:], in0=g_t[:], in1=pa_t[:])
                hr_t = emb.tile([128, H], F32, name="hr_t")
                nc.vector.tensor_add(out=hr_t[:], in0=g_t[:], in1=pr_t[:])
                nc.sync.dma_start(hres[ts(j, 128), :], hr_t[:])

                # LayerNorm statistics (biased var = E[x^2] - E[x]^2)
                sum_t = emb.tile([128, 1], F32, name="sum_t")
                nc.vector.reduce_sum(out=sum_t[:], in_=h_t[:], axis=mybir.AxisListType.X)
                mean_t = emb.tile([128, 1], F32, name="mean_t")
                nc.vector.tensor_scalar_mul(mean_t[:], sum_t[:], 1.0 / H)
                # E[x^2] in one ACT pass: square(h/sqrt(H)) accumulated along free
                sq_t = emb.tile([128, H], BF16, name="sq_t")  # scratch, unused
                ex2_t = emb.tile([128, 1], F32, name="ex2_t")
                nc.scalar.activation(sq_t[:], h_t[:],
                                     mybir.ActivationFunctionType.Square,
                                     scale=float(1.0 / np.sqrt(H)),
                                     accum_out=ex2_t[:])
                msq_t = emb.tile([128, 1], F32, name="msq_t")
                nc.vector.tensor_tensor(out=msq_t[:], in0=mean_t[:], in1=mean_t[:],
                                        op=mybir.AluOpType.mult)
                var_t = emb.tile([128, 1], F32, name="var_t")
                nc.vector.tensor_tensor(out=var_t[:], in0=ex2_t[:], in1=msq_t[:],
                                        op=mybir.AluOpType.subtract)
                std_t = emb.tile([128, 1], F32, name="std_t")
                nc.scalar.activation(std_t[:], var_t[:],
                                     mybir.ActivationFunctionType.Sqrt,
                                     bias=eps_t[:, 0:1])
                rst_t = emb.tile([128, 1], F32, name="rst_t")
                nc.vector.reciprocal(rst_t[:], std_t[:])

                xn_t = emb.tile([128, H], BF16, name="xn_t")
                nc.vector.tensor_scalar(
                    out=xn_t[:], in0=h_t[:],
                    scalar1=mean_t[:, 0:1], scalar2=rst_t[:, 0:1],
                    op0=mybir.AluOpType.subtract, op1=mybir.AluOpType.mult)

                # transpose xn [128tok, H] -> xnT [H, 128tok]
                for kt in range(H // 128):
                    p_t = psA.tile([128, 128], BF16, name="p_t")
                    nc.tensor.transpose(p_t[:], xn_t[:, ts(kt, 128)], ident[:])
                    s_t = emb.tile([128, 128], BF16, name="s_t")
                    nc.any.tensor_copy(out=s_t[:], in_=p_t[:])
                    nc.sync.dma_start(xnT[ts(kt, 128), ts(j, 128)], s_t[:])

        # ---------- Phase B: interT = gelu(w1a.T @ xnT + b1) -----------------
        def g1_reducer(nc_, psum, sbuf, md):
            gi = md.m_tile_idx * md.m_subtiles + md.m_subtile_idx
            out2 = sbuf.rearrange("p a b -> p (a b)")
            nc_.scalar.activation(out2, psum[:, :out2.shape[1]],
                                  mybir.ActivationFunctionType.Gelu_apprx_tanh,
                                  bias=b1_sb[:, gi:gi + 1])

        with ExitStack() as bctx:
            _gemm(bctx, tc, w1a[:], xnT[:], interT[:], output_type=BF16,
                  reducer=g1_reducer, kxm_bufs=6, kxn_bufs=3, psum_bufs=1,
                  tag="g1")

        # ---------- Phase C: comb = interT.T @ w2a + hres --------------------
        hres3 = hres[:].rearrange("(po pi) f -> pi po f", pi=128)  # [128, 2, H]

        with ExitStack() as cctx:
            hrpool = cctx.enter_context(tc.tile_pool(name="hrpool", bufs=3))

            def g2_reducer(nc_, psum, sbuf, md):
                blk = md.m_tile_idx * md.m_subtiles + md.m_subtile_idx
                noff = md.n_tile_idx * md.n_tile + md.n_subtile_idx * md.n_subtile
                nsz = md.n_subtile
                hr = hrpool.tile([128, nsz], F32, name="hr")
                nc_.sync.dma_start(hr[:], hres3[:, blk, ds(noff, nsz)])
                out2 = sbuf.rearrange("p a b -> p (a b)")
                nc_.vector.tensor_tensor(out=out2, in0=psum[:, :nsz], in1=hr[:],
                                         op=mybir.AluOpType.add)

            _gemm(cctx, tc, interT[:], w2a[:], comb[:], output_type=BF16,
                  reducer=g2_reducer, kxm_bufs=17, kxn_bufs=6, tag="g2")

        # ---------- Phase D: logits = comb @ temT ---------------------------
        with ExitStack() as dctx:
            _gemm(dctx, tc, comb[:], temT[:], logits[:], output_type=BF16,
                  transpose_kxm=True, kxm_bufs=3, kxn_bufs=10, temps_bufs=4,
                  tag="g3")

    nc.compile()
    return nc


_NC_CACHE = None


def _get_nc():
    global _NC_CACHE
    if _NC_CACHE is None:
        _NC_CACHE = build_kernel()
    return _NC_CACHE


def _softmax(x):
    e = np.exp(x - x.max())
    return e / e.sum()


def prepare_in_maps(inputs):
    ids = np.asarray(inputs["input_ids"]).astype(np.int32)          # [B, S]
    tok = np.asarray(inputs["tok_emb"], dtype=np.float32)           # [V, H]
    pos = np.asarray(inputs["pos_emb"], dtype=np.float32)[:S]       # [S, H]
    w_router_in = np.asarray(inputs["w_router_in"], dtype=np.float32)
    w_heb = np.asarray(inputs["w_heb"], dtype=np.float32)
    ln_g = np.asarray(inputs["ln_g"], dtype=np.float32)
    ln_b = np.asarray(inputs["ln_b"], dtype=np.float32)
    w1 = np.asarray(inputs["w1"], dtype=np.float32)
    b1 = np.asarray(inputs["b1"], dtype=np.float32)
    w2 = np.asarray(inputs["w2"], dtype=np.float32)
    b2 = np.asarray(inputs["b2"], dtype=np.float32)

    # ---- host-side Hebbian router: selects which expert weights to stage ----
    ids_flat = ids.reshape(-1)                                      # [N_TOK]
    h_mean = tok[ids_flat].mean(axis=0, dtype=np.float64).astype(np.float32)
    h_mean = h_mean + pos.mean(axis=0, dtype=np.float64).astype(np.float32)
    feat = np.tanh(h_mean @ w_router_in)                            # [R]
    scores = w_heb @ feat                                           # [E]
    top_idx = np.argsort(-scores, kind="stable")[:TOPK]
    rw = _softmax(scores[top_idx].astype(np.float32))               # [TOPK]

    # ---- fold LN gain into W1, LN bias into b1, routing weight into W2,
    #      routed b2 into the residual (via pos_resid) ----
    w1s = np.concatenate([ln_g[:, None] * w1[e] for e in top_idx], axis=1)  # [H, KI]
    b1s = np.concatenate([b1[e] + ln_b @ w1[e] for e in top_idx])           # [KI]
    w2s = np.concatenate([rw[k] * w2[e] for k, e in enumerate(top_idx)], axis=0)  # [KI, H]
    bias2 = sum(rw[k] * b2[e] for k, e in enumerate(top_idx))               # [H]

    gtab = tok.astype(BF16_NP)
    temT = np.ascontiguousarray(tok.T).astype(BF16_NP)
    pos_bf = pos.astype(BF16_NP)
    posr_bf = (pos + bias2[None, :]).astype(BF16_NP)
    w1a = w1s.astype(BF16_NP)
    w2a = w2s.astype(BF16_NP)
    b1a = np.ascontiguousarray(b1s.reshape(KI // 128, 128).T)       # [128, KI//128]

    in_maps = []
    for c in range(NCORES):
        tok_slice = ids_flat[c * T:(c + 1) * T]                     # [T]
        ids_pc = np.ascontiguousarray(tok_slice.reshape(T // 128, 128).T)  # [128, 2]
        s0 = (c * T) % S
        in_maps.append({
            "ids": ids_pc,
            "gtab": gtab,
            "pos_a": pos_bf[s0:s0 + T],
            "pos_r": posr_bf[s0:s0 + T],
            "w1a": w1a,
            "b1a": b1a,
            "w2a": w2a,
            "temT": temT,
        })
    return in_maps


def kernel(**inputs) -> np.ndarray:
    nc = _get_nc()
    in_maps = prepare_in_maps(inputs)
    res = run_bass_kernel_spmd(nc, in_maps, core_ids=list(range(NCORES)))
    parts = [np.asarray(res.results[c]["logits"]) for c in range(NCORES)]
    return np.concatenate(parts, axis=0).reshape(B, S, V).astype(np.float32)
